# revision 20
# baseline (speedup 1.0000x reference)
"""Trainium2 Bass kernel for nn_Attention_17222818857675.

Full-input contract: kernel(**inputs) takes the complete tensors, shards
across 8 NeuronCores internally (batch x head-group), runs one SPMD NEFF,
and gathers the full [4, 2048, 1152] output.

Per-core work (b = core//2, g = core%2, heads g*8..g*8+8):
  phase 1: QKV projection in natural [token, dim] layout (bf16 matmuls,
           fp32 PSUM), fused RoPE (bf16 DVE/GpSimd math) + per-head
           RMSNorm (stats pre-RoPE; rotation is norm-preserving),
           PE-transpose of q-hat/k-hat into [dim, token] layout.
  phase 2: per (q-chunk, head): scores S^T = khT.T @ qhT (bf16), softmax
           exp split across engines -- ScalarE exact exp for 5/8 k-tile
           pairs, VectorE Schraudolph bit-trick exp (int16 affine into
           bf16 bits) for 3/8 -- P^T @ V via PE with a leading ones
           column in V giving the denominator for free, normalization
           via reciprocal_approx_fast + gpsimd partition_broadcast,
           then the output projection.  Host sums the two half-head
           partial projections per batch and adds b_proj.
"""

import os
import sys
import types
import numpy as np
import ml_dtypes

# ---------------------------------------------------------------- constants
B, N, C = 4, 2048, 1152
H, DH, HALF = 16, 72, 36
HPC = 8              # heads per core
CPC = HPC * DH       # 576 contraction dims per core
EPS = 1e-6
THETA = 10000.0
NT = N // 128        # 16 token tiles
NCCH = C // 128      # 9 contraction chunks for qkv
QKVC = 432           # qkv output chunk (4 chunks over 1728)
NQKV = (3 * CPC) // QKVC
NJ = 4               # q-chunks of 512
TQ = 512
ECH = 384            # proj output chunk (3 chunks over 1152)
PCB = 5              # proj contraction blocks of 128 (576 -> 4.5 -> 5)

A16 = float(128.0 / np.log(2.0))   # schraudolph bf16 slope
BC16 = 16248.0                     # schraudolph bf16 bias (calibrated)
SCHR = (2, 4, 6)                   # k-tile pairs on the vector-engine exp

_BF16 = ml_dtypes.bfloat16


# ------------------------------------------------------------------- shims
def _install_shims():
    """axon_hooks module (missing in image) + Tile tail-drain walrus fix."""
    try:
        import antenv.axon_hooks  # noqa: F401
    except ImportError:
        import antenv

        m = types.ModuleType("antenv.axon_hooks")
        m._hook = None
        m.set_axon_ntff_profile_hook = lambda h: setattr(m, "_hook", h)
        m.get_axon_ntff_profile_hook = lambda: m._hook
        sys.modules["antenv.axon_hooks"] = m
        antenv.axon_hooks = m
        try:
            from trn_agent_boot.trn_boot import _ntff_profile_via_ctypes

            so = "/opt/axon/libaxon_pjrt.so"
            if os.path.exists(so):
                hook = _ntff_profile_via_ctypes(so)
                if hook:
                    m.set_axon_ntff_profile_hook(hook)
        except Exception:
            pass

    import concourse.tile as tile

    if os.environ.get("BASSK_LDWOPT") == "1":
        import concourse.bass_utils as bu
        if not getattr(bu, "_ldwopt_patched", False):
            import stat, tempfile
            real = bu.get_walrus_driver()
            shim = os.path.join(tempfile.gettempdir(), "walrus_ldwopt.sh")
            with open(shim, "w") as f:
                f.write('#!/bin/bash\nargs=()\nfor a in "$@"; do\n'
                        '  [[ "$a" == "--enable-ldw-opt=false" ]] && a="--enable-ldw-opt=true"\n'
                        '  args+=("$a")\ndone\nexec "%s" "${args[@]}"\n' % real)
            os.chmod(shim, 0o755)
            bu.get_walrus_driver = lambda: shim
            bu._ldwopt_patched = True

    if getattr(tile.TileContext, "_drain_patched", False):
        return

    def _patched(self, tick_clock, wait_clock):
        nc = self.nc
        gc = tick_clock.global_clock
        for proc, sem in self.sems.allocated().items():
            v = gc[proc]
            if v > 0:
                mult = 16 if sem.name.startswith("DMA") else 1
                nc.sync.wait_ge(sem, v * mult)
        nc.sync.drain()
        nc.all_engine_barrier()
        popped = nc._tile_sem_poison_stack.pop()
        assert popped is self._sem_poison
        nc.clear_and_free_semaphores(list(self.sems.allocated().values()))
        nc.all_engine_barrier()

    tile.TileContext._drain_and_barrier = _patched
    tile.TileContext._drain_patched = True


# ------------------------------------------------------------------ builder
_NC = None


def _build():
    global _NC
    if _NC is not None:
        return _NC
    _install_shims()
    import concourse.bass as bass
    import concourse.mybir as mybir
    import concourse.tile as tile
    from concourse import bacc
    from concourse import library_config
    from concourse.masks import make_identity

    f32 = mybir.dt.float32
    i16 = mybir.dt.int16
    bf16 = mybir.dt.bfloat16
    AF = mybir.ActivationFunctionType
    ALU = mybir.AluOpType

    nc = bacc.Bacc(trn_type="TRN2")

    xT_d = nc.dram_tensor("xT", (128, NT, NCCH, 128), bf16, kind="ExternalInput")
    wqkv_d = nc.dram_tensor("wqkv", (128, NCCH, 3 * CPC), bf16, kind="ExternalInput")
    wproj_d = nc.dram_tensor("wproj", (128, PCB, C), bf16, kind="ExternalInput")
    cosq_d = nc.dram_tensor("cosq", (128, NT, DH), f32, kind="ExternalInput")
    sinq_d = nc.dram_tensor("sinq", (128, NT, DH), f32, kind="ExternalInput")
    cosk_d = nc.dram_tensor("cosk", (128, NT, DH), f32, kind="ExternalInput")
    sink_d = nc.dram_tensor("sink", (128, NT, DH), f32, kind="ExternalInput")
    y_d = nc.dram_tensor("y", (N, C), f32, kind="ExternalOutput")

    def APX(base, dims, extra_off=0):
        return bass.AP(tensor=base.tensor, offset=base.offset + extra_off, ap=dims)

    with tile.TileContext(nc) as tc:
        from contextlib import ExitStack

        with ExitStack() as ctx:
            persist = ctx.enter_context(tc.tile_pool(name="persist", bufs=1))
            khT = persist.tile([DH, HPC, N], bf16)           # k-hat transposed
            qT = persist.tile([DH, HPC, N], bf16)            # q-hat transposed
            vaug = persist.tile([128, NT, HPC, 73], bf16)  # ones | 72 v cols
            wqkv = persist.tile([128, NCCH, 3 * CPC], bf16)
            wproj = persist.tile([128, PCB, C], bf16)
            cosq = persist.tile([128, NT, DH], f32)
            sinq = persist.tile([128, NT, DH], f32)
            cosk = persist.tile([128, NT, DH], f32)
            sink = persist.tile([128, NT, DH], f32)
            ident = persist.tile([128, 128], bf16)
            eps_q = persist.tile([128, 1], f32)
            eps_k = persist.tile([128, 1], f32)

            make_identity(nc, ident[:])
            nc.vector.memset(eps_q[:], DH * EPS)
            nc.vector.memset(eps_k[:], EPS)
            nc.vector.memset(vaug[:, :, :, 0:1], 1.0)

            # spread the startup loads over three DMA queues; first halves of
            # the rope tables now, second halves deferred into the tile loop
            HNT = NT // 2
            for t_sb, t_d in ((cosq, cosq_d), (sinq, sinq_d),
                              (cosk, cosk_d), (sink, sink_d)):
                nc.gpsimd.dma_start(t_sb[:, 0:HNT], t_d[:, 0:HNT])

            # ------------------------------------------------ phase 1
            with tc.tile_pool(name="p1", bufs=5) as p1, \
                 tc.tile_pool(name="p1s", bufs=2) as p1s, \
                 tc.tile_pool(name="qkps", bufs=1, space="PSUM") as qkps, \
                 tc.tile_pool(name="trps", bufs=2, space="PSUM") as trps:
                pend = []
                xts = {}
                nc.sync.dma_start(wqkv[:, 0], wqkv_d[:, 0])
                for _p in range(2):
                    xts[_p] = p1.tile([128, NCCH, 128], bf16, tag="xt",
                                      name="xt_pre%d" % _p)
                    nc.sync.dma_start(xts[_p][:], xT_d[:, _p])
                nc.sync.dma_start(wqkv[:, 1], wqkv_d[:, 1])
                for _c in range(2, NCCH):
                    nc.scalar.dma_start(wqkv[:, _c], wqkv_d[:, _c])
                for it in range(NT):
                    if it in xts:
                        xt = xts.pop(it)
                    else:
                        xt = p1.tile([128, NCCH, 128], bf16, tag="xt")
                        nc.sync.dma_start(xt[:], xT_d[:, it])
                    if it == 1:
                        for t_sb, t_d in ((cosq, cosq_d), (sinq, sinq_d),
                                          (cosk, cosk_d), (sink, sink_d)):
                            nc.gpsimd.dma_start(t_sb[:, HNT:NT], t_d[:, HNT:NT])

                    qk = p1.tile([128, 2 * CPC], f32, tag="qk")
                    pss = [qkps.tile([128, QKVC], f32, tag="qkvps%d" % _n,
                                     name="qkvps%d_%d" % (_n, it))
                           for _n in range(NQKV)]
                    # nch-outer: each chunk's PSUM->SBUF copy starts while the
                    # next chunk's matmuls run, freeing its bank for tile it+1
                    for nch in range(NQKV):
                        for cch in range(NCCH):
                            nc.tensor.matmul(
                                pss[nch][:],
                                lhsT=xt[:, cch, :],
                                rhs=wqkv[:, cch, nch * QKVC : (nch + 1) * QKVC],
                                start=(cch == 0),
                                stop=(cch == NCCH - 1),
                            )
                        if nch == 0:
                            nc.scalar.copy(qk[:, 0:QKVC], pss[0][:])
                        elif nch == 1:
                            nc.scalar.copy(qk[:, QKVC : 2 * QKVC], pss[1][:])
                        elif nch == 2:
                            nc.scalar.copy(qk[:, 864:1152], pss[2][:, 0:288])
                            nc.vector.tensor_copy(
                                vaug[:, it, 0:2, 1:73],
                                pss[2][:, 288:432].rearrange(
                                    "p (h d) -> p h d", h=2),
                            )
                        else:
                            nc.vector.tensor_copy(
                                vaug[:, it, 2:8, 1:73],
                                pss[3][:].rearrange("p (h d) -> p h d", h=6),
                            )

                    # RMS stats (pre-RoPE; rotation preserves norms)
                    sq = p1s.tile([128, 2 * CPC], f32, tag="sq")
                    nc.scalar.activation(sq[:], qk[:], AF.Square)
                    ms = p1s.tile([128, 16], f32, tag="ms")
                    nc.vector.tensor_reduce(
                        ms[:], sq[:].rearrange("p (g d) -> p g d", g=16),
                        axis=mybir.AxisListType.X, op=ALU.add,
                    )
                    rms = p1s.tile([128, 16], f32, tag="rms")
                    # q: 1/sqrt(sum + DH*eps) also folds the DH**-0.5 score scale
                    nc.scalar.activation(rms[:, 0:8], ms[:, 0:8], AF.Sqrt, bias=eps_q[:])
                    # k: 1/sqrt(sum/DH + eps)
                    nc.scalar.activation(rms[:, 8:16], ms[:, 8:16], AF.Sqrt,
                                         bias=eps_k[:], scale=1.0 / DH)
                    alpha = p1s.tile([128, 16], f32, tag="alpha")
                    nc.vector.reciprocal(alpha[:], rms[:])

                    # RoPE + alpha scaling.  qk cols: q = [0:576), k = [576:1152)
                    def rope(base_off, cos_t, sin_t, alpha_sl, out_sl, eng):
                        tmp = p1s.tile([128, CPC], f32, tag="ropetmp%d" % base_off)
                        rot = p1s.tile([128, CPC], f32, tag="roterot%d" % base_off)
                        qk0 = qk[:, base_off : base_off + CPC]
                        p_tmp, p_qk = tmp[:].ap[0], qk0.ap[0]
                        p_cos, p_sin = cos_t.ap[0], sin_t.ap[0]
                        p_al, p_out = alpha_sl.ap[0], out_sl.ap[0]
                        # tmp[h,0:36] = x2 * (-sin) ; tmp[h,36:72] = x1 * (+sin)
                        eng.tensor_tensor(
                            APX(tmp[:], [p_tmp, [DH, HPC], [1, HALF]]),
                            APX(qk0, [p_qk, [DH, HPC], [1, HALF]], HALF),
                            APX(sin_t, [p_sin, [0, HPC], [1, HALF]]),
                            op=ALU.mult,
                        )
                        eng.tensor_tensor(
                            APX(tmp[:], [p_tmp, [DH, HPC], [1, HALF]], HALF),
                            APX(qk0, [p_qk, [DH, HPC], [1, HALF]]),
                            APX(sin_t, [p_sin, [0, HPC], [1, HALF]], HALF),
                            op=ALU.mult,
                        )
                        eng.tensor_tensor(
                            rot[:].rearrange("p (h d) -> p h d", h=HPC),
                            qk0.rearrange("p (h d) -> p h d", h=HPC),
                            APX(cos_t, [p_cos, [0, HPC], [1, DH]]),
                            op=ALU.mult,
                        )
                        eng.tensor_tensor(rot[:], rot[:], tmp[:], op=ALU.add)
                        eng.tensor_tensor(
                            out_sl.rearrange("p (h d) -> p h d", h=HPC),
                            rot[:].rearrange("p (h d) -> p h d", h=HPC),
                            APX(alpha_sl, [p_al, [1, HPC], [0, DH]]),
                            op=ALU.mult,
                        )

                    qhat_t = p1s.tile([128, CPC], bf16, tag="qhat")
                    rope(0, cosq[:, it, :], sinq[:, it, :], alpha[:, 0:8],
                         qhat_t[:], nc.vector)
                    khat = p1s.tile([128, CPC], bf16, tag="khat")
                    rope(CPC, cosk[:, it, :], sink[:, it, :], alpha[:, 8:16],
                         khat[:], nc.gpsimd)
                    if len(pend) > 1:
                        emit_ktr(*pend.pop(0))

                    # PE-transpose q-hat/k-hat per head (deferred one tile so
                    # PE never waits on this tile's rope chain)
                    def emit_ktr(it_, khat_, qhat_ref_):
                        for dst, nat, cptag in ((khT, khat_, 0), (qT, qhat_ref_, 1)):
                            tp = trps.tile([DH, HPC, 128], bf16,
                                           tag="ktr%d" % cptag,
                                           name="ktr%d_%d" % (cptag, it_))
                            for hh in range(HPC):
                                nc.tensor.transpose(
                                    tp[:, hh, :],
                                    nat[:, hh * DH : (hh + 1) * DH],
                                    ident[:],
                                )
                            dslice = dst[0:DH, :, it_ * 128 : (it_ + 1) * 128]
                            nc.scalar.copy(dslice, tp[:])
                    pend.append((it, khat, qhat_t))

                for _args in pend:
                    emit_ktr(*_args)

            nc.gpsimd.dma_start(wproj[:], wproj_d[:])
            # proxy library: tensor_tensor + partition_broadcast on gpsimd
            nc.gpsimd.load_library(library_config.proxy)

            # ------------------------------------------------ phase 2
            with tc.tile_pool(name="p2", bufs=2) as p2, \
                 tc.tile_pool(name="p2o", bufs=3) as p2o, \
                 tc.tile_pool(name="sps", bufs=2, space="PSUM") as sps, \
                 tc.tile_pool(name="pvps", bufs=2, space="PSUM") as pvps, \
                 tc.tile_pool(name="yps", bufs=2, space="PSUM") as yps:

                def emit_proj_group(proj_in_, j_, ts, e):
                    yp = yps.tile([128, ECH], f32, tag="yp",
                                  name="yp%d_%d_%d" % (j_, ts, e))
                    for cb in range(PCB):
                        rows = 128 if cb < PCB - 1 else CPC - 128 * (PCB - 1)
                        nc.tensor.matmul(
                            yp[:],
                            lhsT=proj_in_[0:rows, cb, ts * 128 : (ts + 1) * 128],
                            rhs=wproj[0:rows, cb, e * ECH : (e + 1) * ECH],
                            start=(cb == 0), stop=(cb == PCB - 1),
                        )
                    ysb = p2o.tile([128, ECH], f32, tag="ysb",
                                   name="ysb%d_%d_%d" % (j_, ts, e))
                    nc.scalar.copy(ysb[:], yp[:])
                    nc.sync.dma_start(
                        y_d[j_ * TQ + ts * 128 : j_ * TQ + (ts + 1) * 128,
                            e * ECH : (e + 1) * ECH],
                        ysb[:],
                    )

                proj_pend = []
                for j in range(NJ):
                    proj_in = p2.tile([128, PCB, TQ], bf16, tag="proj_in")
                    pvt = {}

                    def flush_pv(h_, g_, pb_):
                        if g_ == 0:
                            pvt[h_] = pvps.tile([73, TQ], f32, tag="pv",
                                                name="pv%d_%d" % (j, h_))
                        pv = pvt[h_]
                        for ii in range(2):
                            i = g_ * 2 + ii
                            nc.tensor.matmul(
                                pv[:],
                                lhsT=vaug[:, i, h_, :],
                                rhs=pb_[:, ii, :],
                                start=(i == 0), stop=(i == 15),
                                skip_group_check=True,
                            )

                    def emit_norm(h_):
                        # pv row 0 is the softmax denominator
                        pv = pvt.pop(h_)
                        nrm = p2o.tile([1, TQ], f32, tag="nrm")
                        nc.vector.reciprocal_approx_fast(nrm[:], pv[0:1, :])
                        bct = p2o.tile([73, TQ], f32, tag="bct")
                        nc.gpsimd.partition_broadcast(bct[:], nrm[:])
                        outT = p2o.tile([73, TQ], bf16, tag="outT")
                        nc.vector.tensor_tensor(outT[:], pv[:], bct[:],
                                                op=ALU.mult)
                        # repack head rows into 128-row proj blocks
                        r0 = h_ * DH
                        cb0, off0 = divmod(r0, 128)
                        n0 = min(DH, 128 - off0)
                        nc.gpsimd.dma_start(
                            proj_in[off0 : off0 + n0, cb0, :], outT[1 : 1 + n0, :]
                        )
                        if n0 < DH:
                            nc.gpsimd.dma_start(
                                proj_in[0 : DH - n0, cb0 + 1, :], outT[1 + n0 : 73, :]
                            )

                    pend_pv = []
                    for h in range(HPC):
                        for g in range(8):
                            sp = sps.tile([128, 2, TQ], f32, tag="sp")
                            for ii in range(2):
                                i = g * 2 + ii
                                nc.tensor.matmul(
                                    sp[:, ii, :],
                                    lhsT=khT[0:DH, h, i * 128 : (i + 1) * 128],
                                    rhs=qT[0:DH, h, j * TQ : (j + 1) * TQ],
                                    start=True, stop=True,
                                )
                            pb = p2o.tile([128, 2, TQ], bf16, tag="pbuf")
                            sp_flat = sp[:].rearrange("p a b -> p (a b)")
                            pb_flat = pb[:].rearrange("p a b -> p (a b)")
                            if g in SCHR:
                                nc.vector.tensor_scalar(
                                    pb_flat.bitcast(i16), sp_flat,
                                    A16, BC16, op0=ALU.mult, op1=ALU.add,
                                )
                            else:
                                nc.scalar.activation(pb_flat, sp_flat, AF.Exp)
                            # two-pair lookahead: PV trails the scores by two
                            pend_pv.append((h, g, pb))
                            if len(pend_pv) > 2:
                                hp, gp, pbp = pend_pv.pop(0)
                                flush_pv(hp, gp, pbp)
                                if gp == 7:
                                    emit_norm(hp)
                                    for _ in range(2):
                                        if proj_pend:
                                            emit_proj_group(*proj_pend.pop(0))
                    for hp, gp, pbp in pend_pv:
                        flush_pv(hp, gp, pbp)
                        if gp == 7:
                            emit_norm(hp)
                            for _ in range(2):
                                if proj_pend:
                                    emit_proj_group(*proj_pend.pop(0))
                    pend_pv = []

                    # queue this q-chunk's projection; drained during next chunk
                    for ts in range(4):
                        for e in range(C // ECH):
                            proj_pend.append((proj_in, j, ts, e))
                for args in proj_pend:
                    emit_proj_group(*args)

    nc.compile()
    _NC = nc
    return nc


# -------------------------------------------------------------- host prep
def _prep_shards(x, w_qkv, w_proj, q_norm_w, k_norm_w):
    inv_freq = 1.0 / (THETA ** (np.arange(HALF, dtype=np.float32) / HALF))
    ang = np.arange(N, dtype=np.float32)[:, None] * inv_freq[None, :]
    cos_t, sin_t = np.cos(ang), np.sin(ang)  # [N, 36]

    def rope_tabs(w):
        # cos2[t, j] = cos(ang) * w[j] (both halves); sin2s = [-sin, +sin] * w
        c2 = np.concatenate([cos_t * w[:HALF], cos_t * w[HALF:]], axis=1)
        s2 = np.concatenate([-sin_t * w[:HALF], sin_t * w[HALF:]], axis=1)
        tile_form = lambda a: np.ascontiguousarray(
            a.reshape(NT, 128, DH).transpose(1, 0, 2)
        ).astype(np.float32)
        return tile_form(c2), tile_form(s2)

    cq, sq_ = rope_tabs(np.asarray(q_norm_w, np.float32))
    ck, sk = rope_tabs(np.asarray(k_norm_w, np.float32))

    xTs = []
    for b in range(B):
        xt = np.ascontiguousarray(x[b].T)  # [1152, 2048]
        xt = xt.reshape(NCCH, 128, NT, 128).transpose(1, 2, 0, 3)
        xTs.append(np.ascontiguousarray(xt).astype(_BF16))

    in_maps = []
    for core in range(8):
        b, g = divmod(core, 2)
        h0 = g * HPC
        rq = w_qkv[h0 * DH : h0 * DH + CPC]                     # [576, 1152]
        rk = w_qkv[C + h0 * DH : C + h0 * DH + CPC]
        rv = w_qkv[2 * C + h0 * DH : 2 * C + h0 * DH + CPC]
        wk = np.concatenate([rq, rk, rv], axis=0).T             # [1152, 1728]
        wk = wk.reshape(NCCH, 128, 3 * CPC).transpose(1, 0, 2)
        wk = np.ascontiguousarray(wk).astype(_BF16)

        wp = w_proj[:, g * CPC : (g + 1) * CPC].T               # [576, 1152]
        wp = np.concatenate(
            [wp, np.zeros((PCB * 128 - CPC, C), np.float32)], axis=0
        )
        wp = wp.reshape(PCB, 128, C).transpose(1, 0, 2)
        wp = np.ascontiguousarray(wp).astype(_BF16)

        in_maps.append({
            "xT": xTs[b], "wqkv": wk, "wproj": wp,
            "cosq": cq, "sinq": sq_, "cosk": ck, "sink": sk,
        })
    return in_maps


def kernel(x, w_qkv, w_proj, b_proj, q_norm_w, k_norm_w):
    x = np.asarray(x, np.float32)
    w_qkv = np.asarray(w_qkv, np.float32)
    w_proj = np.asarray(w_proj, np.float32)
    b_proj = np.asarray(b_proj, np.float32)

    nc = _build()
    from concourse.bass_utils import run_bass_kernel_spmd

    in_maps = _prep_shards(x, w_qkv, w_proj, q_norm_w, k_norm_w)
    res = run_bass_kernel_spmd(nc, in_maps, core_ids=list(range(8)))
    y = np.empty((B, N, C), np.float32)
    for b in range(B):
        y[b] = res.results[2 * b]["y"] + res.results[2 * b + 1]["y"] + b_proj
    return y
